# revision 2
# baseline (speedup 1.0000x reference)
"""KAN layer (piecewise-linear spline) on 8 TRN2 NeuronCores — v3.

Split-relu basis (exact, recentered on the middle grid cell):
  u' = (x+3)/h - 32,  s_j = C_{j+1}-C_j,  gamma_k = s_k - s_{k-1}
  y = sum_f [ C_32*1 + s_31*u' + sum_{k<32} (-gamma_k)*min(u'-(k-32), 0)
                                + sum_{k>=32} gamma_k*relu(u'-(k-32)) ]
Linear extrapolation beyond [-3,3] is exact automatically.  fp16 operands:
recentering keeps tile magnitudes ~|u - 32| so fp16 rounding lands ~1e-2
scale-relative (bf16/uncentered fails).  s/gamma prep is one fp32->fp16 TT
chain on DVE; raw C_32 block is the only cast.

Engines: PE 64 accumulating N=512 fp16 matmuls, col-tiled in two groups
(even slots -> PSUM bank A partitions 0:64, odd -> bank B partitions 64:128,
concurrent on HW); last 10 slots are group-0 only so group 1's merge
(ACT copy -> partition-shift DMA) hides under the final matmuls.
Tiles: DVE (1-op tensor_scalar fp16 4x mode), ACT (Relu with bias, right
knots only), GPSIMD (2-op tensor_scalar).  Coeff streams in 9 chunks on two
HWDGE queues (SP + ACT), s/gamma chunks chase the DMA.
"""

import numpy as np

import concourse.bass as bass
import concourse.mybir as mybir
import concourse.tile as tile
from concourse import bacc
from concourse.bass_utils import run_bass_kernel_spmd

F32 = mybir.dt.float32
F16 = mybir.dt.float16
ALU = mybir.AluOpType
ACTF = mybir.ActivationFunctionType

IN_DIM = 128
OUT_DIM = 64
GRID = 64
B = 4096
N_CORES = 8
BS = B // N_CORES
X_MIN, X_MAX = -3.0, 3.0
H = (X_MAX - X_MIN) / (GRID - 1)
INV_H = 1.0 / H
CENTER = 32
U_OFF = -X_MIN / H - CENTER
N_WARM = 6
CUM = [8, 24, 40, 56, 64]                    # chunk boundaries in grid cols
N_ASYM = 4                                   # trailing group0-only slots


def _tile_engine(k: int) -> str:
    return "gps" if k % 5 in (1, 3) else "dve"


def build_program(reps: int = 1):
    nc = bacc.Bacc(
        "TRN2",
        target_bir_lowering=False,
        debug=False,
        num_devices=N_CORES,
    )
    xT_d = nc.dram_tensor("xT", [IN_DIM, BS], F32, kind="ExternalInput")
    coeff_d = nc.dram_tensor("coeff", [IN_DIM, GRID * OUT_DIM], F32, kind="ExternalInput")
    bias_d = nc.dram_tensor("bias", [1, OUT_DIM], F32, kind="ExternalInput")
    yT_d = nc.dram_tensor("yT", [OUT_DIM, BS], F32, kind="ExternalOutput")

    with tile.TileContext(nc) as tc:
        with (
            tc.tile_pool(name="const", bufs=2) as cpool,
            tc.tile_pool(name="rt", bufs=12) as rpool,
            tc.tile_pool(name="py", bufs=2, space="PSUM") as ppool,
            tc.tile_pool(name="pw", bufs=1, space="PSUM") as wpool,
        ):
            pools = (cpool, rpool, ppool, wpool)
            for rep in range(reps):
                _emit(tc, pools, yT_d.ap(), xT_d.ap(), coeff_d.ap(),
                      bias_d.ap(), warmup=(rep == 0))

    nc.compile()
    return nc


def _emit(tc, pools, yT, xT, coeffR, biasd, warmup=True):
    nc = tc.nc
    cpool, rpool, ppool, wpool = pools

    if True:
        # ---- input DMAs on SP: x first (everything needs u'), then coeff
        xt = cpool.tile([IN_DIM, BS], F32, tag="xt")
        nc.sync.dma_start(out=xt[:], in_=xT[:, :])
        C = cpool.tile([IN_DIM, GRID * OUT_DIM], F32, tag="C")
        lo = 0
        for d, hi_col in enumerate(CUM):
            nc.sync.dma_start(
                out=C[:, lo * OUT_DIM : hi_col * OUT_DIM],
                in_=coeffR[:, lo * OUT_DIM : hi_col * OUT_DIM],
            )
            lo = hi_col
        bt = cpool.tile([1, OUT_DIM], F32, tag="bt")
        nc.sync.dma_start(out=bt[:], in_=biasd[:, :])

        # ---- early DVE work + PE warmup (clock ramp) during first chunks
        if warmup:
            ones = cpool.tile([IN_DIM, BS], F16, tag="ones")
            nc.vector.memset(ones[:], 1.0)
            _emit.ones = ones
            warm = wpool.tile([OUT_DIM, BS], F32, tag="warm")
            for _ in range(N_WARM):
                nc.tensor.matmul(
                    warm[:], ones[:, :OUT_DIM], ones[:], start=True, stop=True
                )
        ones = _emit.ones
        u = cpool.tile([IN_DIM, BS], F16, tag="u")
        nc.vector.tensor_scalar(u[:], xt[:], INV_H, U_OFF, ALU.mult, ALU.add)

        # ---- fp16 staging: chunk casts (ACT), s/gamma chunks (DVE fp16 2x)
        C16 = cpool.tile([IN_DIM, GRID * OUT_DIM], F16, tag="C16")
        s16 = cpool.tile([IN_DIM, (GRID - 1) * OUT_DIM], F16, tag="s16")
        gamL = cpool.tile([IN_DIM, (CENTER - 1) * OUT_DIM], F16, tag="gamL")
        gamR = cpool.tile([IN_DIM, (GRID - CENTER - 1) * OUT_DIM], F16, tag="gamR")

        def cast_block(lo, hi):  # C16 cols [lo, hi)
            nc.scalar.copy(
                out=C16[:, lo * OUT_DIM : hi * OUT_DIM],
                in_=C[:, lo * OUT_DIM : hi * OUT_DIM],
            )

        def s_block(lo, hi):  # s_j = C_{j+1} - C_j for j in [lo, hi)
            nc.vector.tensor_tensor(
                out=s16[:, lo * OUT_DIM : hi * OUT_DIM],
                in0=C16[:, (lo + 1) * OUT_DIM : (hi + 1) * OUT_DIM],
                in1=C16[:, lo * OUT_DIM : hi * OUT_DIM],
                op=ALU.subtract,
            )

        def gam_block(lo, hi):  # gamma_k for k in [lo, hi)
            lo_l, hi_l = max(lo, 1), min(hi, CENTER)
            if lo_l < hi_l:  # left: -gamma_k = s_{k-1} - s_k at col k-1
                nc.vector.tensor_tensor(
                    out=gamL[:, (lo_l - 1) * OUT_DIM : (hi_l - 1) * OUT_DIM],
                    in0=s16[:, (lo_l - 1) * OUT_DIM : (hi_l - 1) * OUT_DIM],
                    in1=s16[:, lo_l * OUT_DIM : hi_l * OUT_DIM],
                    op=ALU.subtract,
                )
            lo_r, hi_r = max(lo, CENTER), min(hi, GRID - 1)
            if lo_r < hi_r:  # right: +gamma_k = s_k - s_{k-1} at col k-32
                nc.vector.tensor_tensor(
                    out=gamR[:, (lo_r - CENTER) * OUT_DIM : (hi_r - CENTER) * OUT_DIM],
                    in0=s16[:, lo_r * OUT_DIM : hi_r * OUT_DIM],
                    in1=s16[:, (lo_r - 1) * OUT_DIM : (hi_r - 1) * OUT_DIM],
                    op=ALU.subtract,
                )

        # ---- MM slots: col-tiled pair accumulation in two PSUM banks
        ypa = ppool.tile([IN_DIM, BS], F32, tag="ypa")
        ypb = ppool.tile([IN_DIM, BS], F32, tag="ypb")
        NSLOT = GRID
        groups = [0 if (i >= NSLOT - N_ASYM or i % 2 == 0) else 1
                  for i in range(NSLOT)]
        n_in_group = [groups.count(0), groups.count(1)]
        seen = [0, 0]
        slot_i = [0]
        g1_done_cb = [None]

        def mm(lhsT, rhs):
            g = groups[slot_i[0]]
            slot_i[0] += 1
            seen[g] += 1
            out = ypa[0:OUT_DIM, :] if g == 0 else ypb[OUT_DIM : 2 * OUT_DIM, :]
            nc.tensor.matmul(
                out, lhsT, rhs,
                start=(seen[g] == 1), stop=(seen[g] == n_in_group[g]),
            )
            if g == 1 and seen[1] == n_in_group[1] and g1_done_cb[0]:
                g1_done_cb[0]()

        def knot_tile(k):
            jp = float(k - CENTER)
            eng = _tile_engine(k)
            r = rpool.tile([IN_DIM, BS], F16, tag="r")
            if k < CENTER:  # tile = min(u'-j', 0); weight -gamma
                op1 = ALU.min
                w = gamL[:, (k - 1) * OUT_DIM : k * OUT_DIM]
            else:          # tile = relu(u'-j'); weight +gamma
                op1 = ALU.max
                w = gamR[:, (k - CENTER) * OUT_DIM : (k - CENTER + 1) * OUT_DIM]
            if eng == "gps":
                nc.gpsimd.tensor_scalar(r[:], u[:], jp, 0.0, ALU.subtract, op1)
            else:
                nc.vector.tensor_scalar(r[:], u[:], jp, 0.0, ALU.subtract, op1)
            mm(w, r[:])

        # merge step 1, fired right after group1's last matmul is emitted:
        # copy bank-B partial to SBUF and write it straight to DRAM — the
        # bank-A partial is then DMA-accumulated on top (SWDGE accum add).
        hi = cpool.tile([IN_DIM, BS], F32, tag="hi")

        def g1_merge():
            nc.scalar.copy(
                out=hi[OUT_DIM : 2 * OUT_DIM, :],
                in_=ypb[OUT_DIM : 2 * OUT_DIM, :],
            )
            nc.gpsimd.dma_start(out=yT[:, :], in_=hi[OUT_DIM : 2 * OUT_DIM, :])

        g1_done_cb[0] = g1_merge

        # ---- chunk-chasing pipeline over knots k = 1..62
        done_k = 0
        cast_lo = 0
        for c, hi_col in enumerate(CUM):
            cast_block(cast_lo, hi_col)
            cast_lo = hi_col
            if c + 1 < len(CUM):
                s_hi = hi_col - 1          # s needs C_{j+1}
            else:
                s_hi = GRID - 1
            s_lo = CUM[c - 1] - 1 if c > 0 else 0
            if s_lo < s_hi:
                s_block(s_lo, s_hi)
            g_hi = min(s_hi, GRID - 1)     # gammas up to s_hi - 1 index
            gam_block(done_k + 1, g_hi)
            for k in range(done_k + 1, g_hi):
                knot_tile(k)
            done_k = g_hi - 1
        for k in range(done_k + 1, GRID - 1):
            knot_tile(k)

        # ---- linear + constant slots (weights ready since mid-stream)
        W1 = cpool.tile([IN_DIM, OUT_DIM], F16, tag="W1")
        nc.vector.tensor_copy(
            W1[:], C16[:, CENTER * OUT_DIM : (CENTER + 1) * OUT_DIM]
        )
        bt16 = cpool.tile([1, OUT_DIM], F16, tag="bt16")
        nc.scalar.copy(out=bt16[:], in_=bt[:])
        nc.vector.tensor_tensor(
            out=W1[0:1, :], in0=W1[0:1, :], in1=bt16[:], op=ALU.add
        )
        mm(s16[:, (CENTER - 1) * OUT_DIM : CENTER * OUT_DIM], u[:])   # slope
        mm(W1[:], ones[:])                                            # const

        # ---- final: bank-A partial -> SBUF, DMA-accumulate onto yT
        yt = cpool.tile([OUT_DIM, BS], F32, tag="yt")
        nc.scalar.copy(out=yt[:], in_=ypa[0:OUT_DIM, :])
        nc.gpsimd.dma_start(out=yT[:, :], in_=yt[:], accum_op=ALU.add)


_NC_CACHE = {}


def _get_program():
    if "nc" not in _NC_CACHE:
        _NC_CACHE["nc"] = build_program()
    return _NC_CACHE["nc"]


def make_in_maps(x, coeff, bias):
    x = np.ascontiguousarray(np.asarray(x, dtype=np.float32))
    coeff_r = np.ascontiguousarray(
        np.asarray(coeff, dtype=np.float32).reshape(IN_DIM, GRID * OUT_DIM)
    )
    bias_r = np.ascontiguousarray(
        np.asarray(bias, dtype=np.float32).reshape(1, OUT_DIM)
    )
    in_maps = []
    for c in range(N_CORES):
        xs = np.ascontiguousarray(x[c * BS : (c + 1) * BS, :].T)
        in_maps.append({"xT": xs, "coeff": coeff_r, "bias": bias_r})
    return in_maps


def kernel(x, coeff, bias):
    nc = _get_program()
    in_maps = make_in_maps(x, coeff, bias)
    res = run_bass_kernel_spmd(nc, in_maps, list(range(N_CORES)))
    y = np.concatenate([r["yT"].T for r in res.results], axis=0)
    return np.ascontiguousarray(y.astype(np.float32))


if __name__ == "__main__":
    xx = np.random.randn(B, IN_DIM).astype(np.float32)
    cc = (np.random.randn(IN_DIM, GRID, OUT_DIM) * 0.02).astype(np.float32)
    bb = np.zeros(OUT_DIM, dtype=np.float32)
    yy = kernel(xx, cc, bb)
    print("kernel output:", yy.shape, yy.dtype, float(np.abs(yy).mean()))
